# revision 1
# baseline (speedup 1.0000x reference)
"""GQA attention kernel for Trainium2, sharded over 8 NeuronCores.

Problem: B=2, S=2048, D=2048, H=16 query heads, KV=4 kv heads, HD=128,
RoPE, no causal mask, out = softmax(q k^T / sqrt(HD)) v @ Wo.

Sharding: core = b*4 + g  (b in {0,1} batch, g in {0..3} head group).
Each core handles 4 query heads [4g..4g+3] and kv head g (exact GQA
split), with Wq/Wk/Wv column-sliced and Wo row-sliced.  Each core
produces a partial o_proj output for its batch; host sums the 4 partials
per batch.

Per-core layout strategy (all matmuls bf16 with fp32 PSUM accumulation):
  - host supplies h[b]^T (so contraction dim D is on partitions)
  - q/k/v projections -> [s, cols] tiles; RoPE applied in fp32
  - q_rot/k_rot PE-transposed to qT/kT [HD, S]
  - scores^T tile = matmul(lhsT=kT_tile, rhs=qT_block)   (K=HD=128)
  - exp on ScalarE straight out of PSUM (no max subtraction: scores are
    ~N(0,1) with |s| < ~6.5, safely inside fp32/exp range)
  - out^T = sum_c matmul(lhsT=v_chunk, rhs=expT_chunk)
  - denominators via matmul(lhsT=ones[128,1], rhs=expT_chunk)
  - normalize with reciprocal broadcast across partitions (GpSimd)
  - o_proj: y = sum_ch matmul(lhsT=attn_outT chunk, rhs=Wo chunk) ->
    natural [s, dout] layout, DMA to HBM fp32
"""

import math
import numpy as np
import ml_dtypes

B, S, D = 2, 2048, 2048
H, KV, HD = 16, 4, 128
G = 4          # tensor-parallel head groups
HG = H // G    # 4 query heads per core
QCOLS = HG * HD  # 512
P = 128
NT = S // P    # 16 sequence tiles
KO = D // P    # 16 contraction chunks
NB = S // 512  # 4 query blocks of 512

BF16 = ml_dtypes.bfloat16

_CACHE = {}


def _build_nc():
    import concourse.mybir as mybir
    import concourse.tile as tile
    from concourse import bacc
    from concourse.masks import make_identity
    from contextlib import ExitStack

    dt = mybir.dt
    nc = bacc.Bacc(
        "TRN2",
        target_bir_lowering=False,
        debug=False,
        enable_asserts=False,
        num_devices=8,
    )

    hT = nc.dram_tensor("hT", [D, S], dt.bfloat16, kind="ExternalInput").ap()
    wq = nc.dram_tensor("wq", [D, QCOLS], dt.bfloat16, kind="ExternalInput").ap()
    wk = nc.dram_tensor("wk", [D, HD], dt.bfloat16, kind="ExternalInput").ap()
    wv = nc.dram_tensor("wv", [D, HD], dt.bfloat16, kind="ExternalInput").ap()
    wo = nc.dram_tensor("wo", [QCOLS, D], dt.bfloat16, kind="ExternalInput").ap()
    cosd = nc.dram_tensor("cosd", [S, HD], dt.float32, kind="ExternalInput").ap()
    sind = nc.dram_tensor("sind", [S, HD], dt.float32, kind="ExternalInput").ap()
    y = nc.dram_tensor("y", [S, D], dt.float32, kind="ExternalOutput").ap()

    with tile.TileContext(nc) as tc:
        _emit(tc, nc, mybir, hT, wq, wk, wv, wo, cosd, sind, y, make_identity)

    nc.compile()
    return nc


def _emit(tc, nc, mybir, hT, wq, wk, wv, wo, cosd, sind, y, make_identity):
    from contextlib import ExitStack

    dt = mybir.dt
    bf16 = dt.bfloat16
    f32 = dt.float32
    Exp = mybir.ActivationFunctionType.Exp

    with ExitStack() as ctx:
        const = ctx.enter_context(tc.tile_pool(name="const", bufs=1))
        wpool = ctx.enter_context(tc.tile_pool(name="wpool", bufs=1))
        big = ctx.enter_context(tc.tile_pool(name="big", bufs=1))
        hpool = ctx.enter_context(tc.tile_pool(name="hpool", bufs=3))
        work = ctx.enter_context(tc.tile_pool(name="work", bufs=3))
        expp = ctx.enter_context(tc.tile_pool(name="expp", bufs=4))
        ps_big = ctx.enter_context(tc.tile_pool(name="ps_big", bufs=4, space="PSUM"))
        ps_small = ctx.enter_context(
            tc.tile_pool(name="ps_small", bufs=3, space="PSUM")
        )
        ps_sum = ctx.enter_context(tc.tile_pool(name="ps_sum", bufs=1, space="PSUM"))

        # --- constants ---
        ident = const.tile([P, P], bf16)
        make_identity(nc, ident)
        ones = const.tile([P, 1], bf16)
        nc.vector.memset(ones, 1.0)

        # --- weights and tables to SBUF ---
        wq_sb = wpool.tile([P, KO, QCOLS], bf16)
        nc.sync.dma_start(wq_sb, wq.rearrange("(ko p) m -> p ko m", p=P))
        wk_sb = wpool.tile([P, KO, HD], bf16)
        nc.sync.dma_start(wk_sb, wk.rearrange("(ko p) m -> p ko m", p=P))
        wv_sb = wpool.tile([P, KO, HD], bf16)
        nc.sync.dma_start(wv_sb, wv.rearrange("(ko p) m -> p ko m", p=P))
        wo_sb = wpool.tile([P, HG, D], bf16)
        nc.sync.dma_start(wo_sb, wo.rearrange("(ch p) n -> p ch n", p=P))
        cos_sb = wpool.tile([P, NT, HD], f32)
        nc.sync.dma_start(cos_sb, cosd.rearrange("(i p) c -> p i c", p=P))
        sin_sb = wpool.tile([P, NT, HD], f32)
        nc.sync.dma_start(sin_sb, sind.rearrange("(i p) c -> p i c", p=P))

        # --- persistent intermediates ---
        qT = big.tile([P, HG, S], bf16)    # [hd, head, s]
        kT = big.tile([P, S], bf16)        # [hd, s]
        v_sb = big.tile([P, NT, HD], bf16)  # [s_inner, s_chunk, hd]
        aoT = big.tile([P, HG, S], bf16)   # attn_out^T  [c_inner, head, s]

        hT_r = hT.rearrange("(ko p) s -> p ko s", p=P)

        # ---------------- Phase 1: QKV projections + RoPE + transposes ------
        for i in range(NT):
            hT_t = hpool.tile([P, KO, P], bf16, tag="ht")
            nc.sync.dma_start(hT_t, hT_r[:, :, i * P : (i + 1) * P])

            ps_q = ps_big.tile([P, 512], f32, tag="big")
            ps_k = ps_small.tile([P, HD], f32, tag="small")
            ps_v = ps_small.tile([P, HD], f32, tag="small")
            for ko in range(KO):
                first, last = ko == 0, ko == KO - 1
                nc.tensor.matmul(
                    ps_q, hT_t[:, ko], wq_sb[:, ko], start=first, stop=last
                )
                nc.tensor.matmul(
                    ps_k, hT_t[:, ko], wk_sb[:, ko], start=first, stop=last
                )
                nc.tensor.matmul(
                    ps_v, hT_t[:, ko], wv_sb[:, ko], start=first, stop=last
                )

            # v: straight cast copy into [s, hd] layout
            nc.scalar.copy(v_sb[:, i], ps_v)

            # q/k to fp32 SBUF for RoPE
            q_f = work.tile([P, QCOLS], f32, tag="qf")
            nc.scalar.copy(q_f, ps_q)
            k_f = work.tile([P, HD], f32, tag="kf")
            nc.scalar.copy(k_f, ps_k)

            cos_t = cos_sb[:, i]
            sin_t = sin_sb[:, i]
            q_rot = work.tile([P, QCOLS], bf16, tag="qrot")
            k_rot = work.tile([P, HD], bf16, tag="krot")

            def rope(dst, src, ncols):
                for h0 in range(0, ncols, HD):
                    lo = slice(h0, h0 + 64)
                    hi = slice(h0 + 64, h0 + HD)
                    t1 = work.tile([P, 64], f32, tag="rt1")
                    t2 = work.tile([P, 64], f32, tag="rt2")
                    # dst_lo = src_lo*cos_lo - src_hi*sin_lo
                    nc.vector.tensor_mul(t1, src[:, lo], cos_t[:, 0:64])
                    nc.vector.tensor_mul(t2, src[:, hi], sin_t[:, 0:64])
                    nc.vector.tensor_sub(dst[:, lo], t1, t2)
                    # dst_hi = src_hi*cos_hi + src_lo*sin_hi
                    t3 = work.tile([P, 64], f32, tag="rt1")
                    t4 = work.tile([P, 64], f32, tag="rt2")
                    nc.vector.tensor_mul(t3, src[:, hi], cos_t[:, 64:HD])
                    nc.vector.tensor_mul(t4, src[:, lo], sin_t[:, 64:HD])
                    nc.vector.tensor_add(dst[:, hi], t3, t4)

            rope(q_rot, q_f, QCOLS)
            rope(k_rot, k_f, HD)

            # transpose q_rot/k_rot 128x128 tiles -> qT/kT
            for h in range(HG):
                ps_t = ps_small.tile([P, P], bf16, tag="small")
                nc.tensor.transpose(ps_t, q_rot[:, h * HD : (h + 1) * HD], ident)
                nc.vector.tensor_copy(qT[:, h, i * P : (i + 1) * P], ps_t)
            ps_t2 = ps_small.tile([P, P], bf16, tag="small")
            nc.tensor.transpose(ps_t2, k_rot, ident)
            nc.vector.tensor_copy(kT[:, i * P : (i + 1) * P], ps_t2)

        # ---------------- Phase 2: attention --------------------------------
        for b in range(NB):
            qs = slice(b * 512, (b + 1) * 512)
            for h in range(HG):
                ps_o = ps_big.tile([P, 512], f32, tag="big")
                ps_sm = ps_sum.tile([1, 512], f32, tag="sum")
                for c in range(NT):
                    first, last = c == 0, c == NT - 1
                    ps_s = ps_big.tile([P, 512], f32, tag="big")
                    nc.tensor.matmul(
                        ps_s,
                        kT[:, c * P : (c + 1) * P],
                        qT[:, h, qs],
                        start=True,
                        stop=True,
                    )
                    expT = expp.tile([P, 512], bf16, tag="exp")
                    nc.scalar.activation(expT, ps_s, Exp)
                    nc.tensor.matmul(ps_o, v_sb[:, c], expT, start=first, stop=last)
                    nc.tensor.matmul(ps_sm, ones, expT, start=first, stop=last)

                sums_f = work.tile([1, 512], f32, tag="sums")
                nc.vector.tensor_copy(sums_f, ps_sm)
                recip = work.tile([1, 512], f32, tag="recip")
                nc.vector.reciprocal(recip, sums_f)
                recip_bc = work.tile([P, 512], f32, tag="rbc")
                nc.gpsimd.partition_broadcast(recip_bc, recip)
                nc.vector.tensor_mul(aoT[:, h, qs], ps_o, recip_bc)

        # ---------------- Phase 3: o_proj -----------------------------------
        y_r = y.rearrange("(i p) n -> p i n", p=P)
        for i in range(NT):
            for nb in range(NB):
                ns = slice(nb * 512, (nb + 1) * 512)
                ps_y = ps_big.tile([P, 512], f32, tag="big")
                for ch in range(HG):
                    nc.tensor.matmul(
                        ps_y,
                        aoT[:, ch, i * P : (i + 1) * P],
                        wo_sb[:, ch, ns],
                        start=(ch == 0),
                        stop=(ch == HG - 1),
                    )
                y_sb = work.tile([P, 512], f32, tag="ysb")
                nc.vector.tensor_copy(y_sb, ps_y)
                nc.sync.dma_start(y_r[:, i, ns], y_sb)


def get_nc():
    if "nc" not in _CACHE:
        _CACHE["nc"] = _build_nc()
    return _CACHE["nc"]


def make_in_maps(inputs):
    """Shard full inputs into 8 per-core input maps."""
    h = np.asarray(inputs["hidden_states"], dtype=np.float32)
    cos = np.asarray(inputs["cos"], dtype=np.float32).reshape(S, HD)
    sin = np.asarray(inputs["sin"], dtype=np.float32).reshape(S, HD)
    # fold the 1/sqrt(HD) softmax scale into Wq before the bf16 cast
    Wq = np.asarray(inputs["Wq"], dtype=np.float32) * (HD ** -0.5)
    Wk = np.asarray(inputs["Wk"], dtype=np.float32)
    Wv = np.asarray(inputs["Wv"], dtype=np.float32)
    Wo = np.asarray(inputs["Wo"], dtype=np.float32)

    hT = [np.ascontiguousarray(h[b].T).astype(BF16) for b in range(B)]
    wq_s = [np.ascontiguousarray(Wq[:, g * QCOLS : (g + 1) * QCOLS]).astype(BF16) for g in range(G)]
    wk_s = [np.ascontiguousarray(Wk[:, g * HD : (g + 1) * HD]).astype(BF16) for g in range(G)]
    wv_s = [np.ascontiguousarray(Wv[:, g * HD : (g + 1) * HD]).astype(BF16) for g in range(G)]
    wo_s = [np.ascontiguousarray(Wo[g * QCOLS : (g + 1) * QCOLS, :]).astype(BF16) for g in range(G)]

    in_maps = []
    for core in range(8):
        b, g = divmod(core, G)
        in_maps.append(
            {
                "hT": hT[b],
                "wq": wq_s[g],
                "wk": wk_s[g],
                "wv": wv_s[g],
                "wo": wo_s[g],
                "cosd": cos,
                "sind": sin,
            }
        )
    return in_maps


def kernel(**inputs) -> np.ndarray:
    from concourse import bass_utils

    nc = get_nc()
    in_maps = make_in_maps(inputs)
    res = bass_utils.run_bass_kernel_spmd(nc, in_maps, core_ids=list(range(8)))
    out = np.zeros((B, S, D), dtype=np.float32)
    for core in range(8):
        b = core // G
        out[b] += res.results[core]["y"]
    return out


# revision 5
# speedup vs baseline: 1.0516x; 1.0516x over previous
"""GQA attention kernel for Trainium2, sharded over 8 NeuronCores.

Problem: B=2, S=2048, D=2048, H=16 query heads, KV=4 kv heads, HD=128,
RoPE, no causal mask, out = softmax(q k^T / sqrt(HD)) v @ Wo.

Sharding: core = b*4 + g  (b in {0,1} batch, g in {0..3} head group).
Each core handles 4 query heads [4g..4g+3] and kv head g (exact GQA
split), with Wq/Wk/Wv column-sliced and Wo row-sliced.  Each core
produces a partial o_proj output for its batch; host sums the 4 partials
per batch.

Per-core layout strategy (all matmuls bf16 with fp32 PSUM accumulation):
  - host supplies h[b]^T (so contraction dim D is on partitions)
  - q/k/v projections -> [s, cols] tiles; RoPE applied in fp32
  - q_rot/k_rot PE-transposed to qT/kT [HD, S]
  - scores^T tile = matmul(lhsT=kT_tile, rhs=qT_block)   (K=HD=128)
  - exp on ScalarE straight out of PSUM (no max subtraction: scores are
    ~N(0,1) with |s| < ~6.5, safely inside fp32/exp range)
  - out^T = sum_c matmul(lhsT=v_chunk, rhs=expT_chunk)
  - denominators via matmul(lhsT=ones[128,1], rhs=expT_chunk)
  - normalize with reciprocal broadcast across partitions (GpSimd)
  - o_proj: y = sum_ch matmul(lhsT=attn_outT chunk, rhs=Wo chunk) ->
    natural [s, dout] layout, DMA to HBM fp32
"""

import math
import numpy as np
import ml_dtypes

B, S, D = 2, 2048, 2048
H, KV, HD = 16, 4, 128
G = 4          # tensor-parallel head groups
HG = H // G    # 4 query heads per core
QCOLS = HG * HD  # 512
P = 128
NT = S // P    # 16 sequence tiles
KO = D // P    # 16 contraction chunks
NB = S // 512  # 4 query blocks of 512

BF16 = ml_dtypes.bfloat16

_CACHE = {}


def _build_nc():
    import concourse.mybir as mybir
    import concourse.tile as tile
    from concourse import bacc
    from concourse.masks import make_identity
    from contextlib import ExitStack

    dt = mybir.dt
    nc = bacc.Bacc(
        "TRN2",
        target_bir_lowering=False,
        debug=False,
        enable_asserts=False,
        num_devices=8,
    )

    hT = nc.dram_tensor("hT", [D, S], dt.bfloat16, kind="ExternalInput").ap()
    wq = nc.dram_tensor("wq", [D, QCOLS], dt.bfloat16, kind="ExternalInput").ap()
    wk = nc.dram_tensor("wk", [D, HD], dt.bfloat16, kind="ExternalInput").ap()
    wv = nc.dram_tensor("wv", [D, HD], dt.bfloat16, kind="ExternalInput").ap()
    wo = nc.dram_tensor("wo", [QCOLS, D], dt.bfloat16, kind="ExternalInput").ap()
    cosd = nc.dram_tensor("cosd", [S, HD], dt.float32, kind="ExternalInput").ap()
    sind = nc.dram_tensor("sind", [S, HD], dt.float32, kind="ExternalInput").ap()
    y = nc.dram_tensor("y", [S, D], dt.float32, kind="ExternalOutput").ap()

    with tile.TileContext(nc) as tc:
        _emit(tc, nc, mybir, hT, wq, wk, wv, wo, cosd, sind, y, make_identity)

    nc.compile()
    return nc


def _emit(tc, nc, mybir, hT, wq, wk, wv, wo, cosd, sind, y, make_identity):
    from contextlib import ExitStack

    dt = mybir.dt
    bf16 = dt.bfloat16
    f32 = dt.float32
    Exp = mybir.ActivationFunctionType.Exp

    with ExitStack() as ctx:
        const = ctx.enter_context(tc.tile_pool(name="const", bufs=1))
        wpool = ctx.enter_context(tc.tile_pool(name="wpool", bufs=1))
        big = ctx.enter_context(tc.tile_pool(name="big", bufs=1))
        hpool = ctx.enter_context(tc.tile_pool(name="hpool", bufs=4))
        work = ctx.enter_context(tc.tile_pool(name="work", bufs=4))
        expp = ctx.enter_context(tc.tile_pool(name="expp", bufs=6))
        ps_big = ctx.enter_context(tc.tile_pool(name="ps_big", bufs=4, space="PSUM"))
        ps_small = ctx.enter_context(
            tc.tile_pool(name="ps_small", bufs=3, space="PSUM")
        )
        ps_sum = ctx.enter_context(tc.tile_pool(name="ps_sum", bufs=1, space="PSUM"))

        # --- constants ---
        ident = const.tile([P, P], bf16)
        make_identity(nc, ident)
        ones = const.tile([P, 1], bf16)
        nc.vector.memset(ones, 1.0)

        # --- weights and tables to SBUF ---
        wq_sb = wpool.tile([P, KO, QCOLS], bf16)
        nc.sync.dma_start(wq_sb, wq.rearrange("(ko p) m -> p ko m", p=P))
        # k and v weights interleaved into one [P, KO, 256] tile so the k and
        # v projections run as a single N=256 matmul per contraction chunk
        wkv_sb = wpool.tile([P, KO, 2 * HD], bf16)
        nc.sync.dma_start(
            wkv_sb[:, :, :HD], wk.rearrange("(ko p) m -> p ko m", p=P)
        )
        nc.sync.dma_start(
            wkv_sb[:, :, HD:], wv.rearrange("(ko p) m -> p ko m", p=P)
        )
        wo_sb = wpool.tile([P, HG, D], bf16)
        nc.sync.dma_start(wo_sb, wo.rearrange("(ch p) n -> p ch n", p=P))
        cos_sb = wpool.tile([P, NT, HD], f32)
        nc.sync.dma_start(cos_sb, cosd.rearrange("(i p) c -> p i c", p=P))
        sin_sb = wpool.tile([P, NT, HD], f32)
        nc.sync.dma_start(sin_sb, sind.rearrange("(i p) c -> p i c", p=P))

        # --- persistent intermediates ---
        qT = big.tile([P, HG, S], bf16)    # [hd, head, s]
        kT = big.tile([P, S], bf16)        # [hd, s]
        v_sb = big.tile([P, NT, HD], bf16)  # [s_inner, s_chunk, hd]
        aoT = big.tile([P, HG, S], bf16)   # attn_out^T  [c_inner, head, s]

        hT_r = hT.rearrange("(ko p) s -> p ko s", p=P)

        # ---------------- Phase 1: QKV projections + RoPE + transposes ------
        for i in range(NT):
            hT_t = hpool.tile([P, KO, P], bf16, tag="ht")
            nc.sync.dma_start(hT_t, hT_r[:, :, i * P : (i + 1) * P])

            ps_q = ps_big.tile([P, 512], f32, tag="big")
            ps_kv = ps_small.tile([P, 2 * HD], f32, tag="small")
            for ko in range(KO):
                first, last = ko == 0, ko == KO - 1
                nc.tensor.matmul(
                    ps_q, hT_t[:, ko], wq_sb[:, ko], start=first, stop=last
                )
                nc.tensor.matmul(
                    ps_kv, hT_t[:, ko], wkv_sb[:, ko], start=first, stop=last
                )

            # v: straight cast copy into [s, hd] layout
            nc.scalar.copy(v_sb[:, i], ps_kv[:, HD:])

            # q/k to fp32 SBUF for RoPE
            q_f = work.tile([P, QCOLS], f32, tag="qf")
            nc.scalar.copy(q_f, ps_q)
            k_f = work.tile([P, HD], f32, tag="kf")
            nc.scalar.copy(k_f, ps_kv[:, :HD])

            cos_t = cos_sb[:, i]
            sin_t = sin_sb[:, i]
            q_rot = work.tile([P, QCOLS], bf16, tag="qrot")
            k_rot = work.tile([P, HD], bf16, tag="krot")

            def rope(dst, src, ncols):
                for h0 in range(0, ncols, HD):
                    lo = slice(h0, h0 + 64)
                    hi = slice(h0 + 64, h0 + HD)
                    t1 = work.tile([P, 64], f32, tag="rt1")
                    t2 = work.tile([P, 64], f32, tag="rt2")
                    # dst_lo = src_lo*cos_lo - src_hi*sin_lo
                    nc.vector.tensor_mul(t1, src[:, lo], cos_t[:, 0:64])
                    nc.vector.tensor_mul(t2, src[:, hi], sin_t[:, 0:64])
                    nc.vector.tensor_sub(dst[:, lo], t1, t2)
                    # dst_hi = src_hi*cos_hi + src_lo*sin_hi
                    t3 = work.tile([P, 64], f32, tag="rt1")
                    t4 = work.tile([P, 64], f32, tag="rt2")
                    nc.vector.tensor_mul(t3, src[:, hi], cos_t[:, 64:HD])
                    nc.vector.tensor_mul(t4, src[:, lo], sin_t[:, 64:HD])
                    nc.vector.tensor_add(dst[:, hi], t3, t4)

            rope(q_rot, q_f, QCOLS)
            rope(k_rot, k_f, HD)

            # transpose q_rot/k_rot 128x128 tiles -> qT/kT
            for h in range(HG):
                ps_t = ps_small.tile([P, P], bf16, tag="small")
                nc.tensor.transpose(ps_t, q_rot[:, h * HD : (h + 1) * HD], ident)
                nc.vector.tensor_copy(qT[:, h, i * P : (i + 1) * P], ps_t)
            ps_t2 = ps_small.tile([P, P], bf16, tag="small")
            nc.tensor.transpose(ps_t2, k_rot, ident)
            nc.vector.tensor_copy(kT[:, i * P : (i + 1) * P], ps_t2)

        # ---------------- Phase 2: attention --------------------------------
        for b in range(NB):
            qs = slice(b * 512, (b + 1) * 512)
            for h in range(HG):
                ps_o = ps_big.tile([P, 512], f32, tag="big")
                ps_sm = ps_sum.tile([1, 512], f32, tag="sum")
                for c in range(NT):
                    first, last = c == 0, c == NT - 1
                    ps_s = ps_big.tile([P, 512], f32, tag="big")
                    nc.tensor.matmul(
                        ps_s,
                        kT[:, c * P : (c + 1) * P],
                        qT[:, h, qs],
                        start=True,
                        stop=True,
                    )
                    expT = expp.tile([P, 512], bf16, tag="exp")
                    nc.scalar.activation(expT, ps_s, Exp)
                    nc.tensor.matmul(ps_o, v_sb[:, c], expT, start=first, stop=last)
                    nc.tensor.matmul(ps_sm, ones, expT, start=first, stop=last)

                sums_f = work.tile([1, 512], f32, tag="sums")
                nc.vector.tensor_copy(sums_f, ps_sm)
                recip = work.tile([1, 512], f32, tag="recip")
                nc.vector.reciprocal(recip, sums_f)
                recip_bc = work.tile([P, 512], f32, tag="rbc")
                nc.gpsimd.partition_broadcast(recip_bc, recip)
                nc.vector.tensor_mul(aoT[:, h, qs], ps_o, recip_bc)

        # ---------------- Phase 3: o_proj -----------------------------------
        y_r = y.rearrange("(i p) n -> p i n", p=P)
        for i in range(NT):
            for nb in range(NB):
                ns = slice(nb * 512, (nb + 1) * 512)
                ps_y = ps_big.tile([P, 512], f32, tag="big")
                for ch in range(HG):
                    nc.tensor.matmul(
                        ps_y,
                        aoT[:, ch, i * P : (i + 1) * P],
                        wo_sb[:, ch, ns],
                        start=(ch == 0),
                        stop=(ch == HG - 1),
                    )
                y_sb = work.tile([P, 512], f32, tag="ysb")
                nc.scalar.copy(y_sb, ps_y)
                nc.sync.dma_start(y_r[:, i, ns], y_sb)


def get_nc():
    if "nc" not in _CACHE:
        _CACHE["nc"] = _build_nc()
    return _CACHE["nc"]


def make_in_maps(inputs):
    """Shard full inputs into 8 per-core input maps."""
    h = np.asarray(inputs["hidden_states"], dtype=np.float32)
    cos = np.asarray(inputs["cos"], dtype=np.float32).reshape(S, HD)
    sin = np.asarray(inputs["sin"], dtype=np.float32).reshape(S, HD)
    # fold the 1/sqrt(HD) softmax scale into Wq before the bf16 cast
    Wq = np.asarray(inputs["Wq"], dtype=np.float32) * (HD ** -0.5)
    Wk = np.asarray(inputs["Wk"], dtype=np.float32)
    Wv = np.asarray(inputs["Wv"], dtype=np.float32)
    Wo = np.asarray(inputs["Wo"], dtype=np.float32)

    hT = [np.ascontiguousarray(h[b].T).astype(BF16) for b in range(B)]
    wq_s = [np.ascontiguousarray(Wq[:, g * QCOLS : (g + 1) * QCOLS]).astype(BF16) for g in range(G)]
    wk_s = [np.ascontiguousarray(Wk[:, g * HD : (g + 1) * HD]).astype(BF16) for g in range(G)]
    wv_s = [np.ascontiguousarray(Wv[:, g * HD : (g + 1) * HD]).astype(BF16) for g in range(G)]
    wo_s = [np.ascontiguousarray(Wo[g * QCOLS : (g + 1) * QCOLS, :]).astype(BF16) for g in range(G)]

    in_maps = []
    for core in range(8):
        b, g = divmod(core, G)
        in_maps.append(
            {
                "hT": hT[b],
                "wq": wq_s[g],
                "wk": wk_s[g],
                "wv": wv_s[g],
                "wo": wo_s[g],
                "cosd": cos,
                "sind": sin,
            }
        )
    return in_maps


def kernel(**inputs) -> np.ndarray:
    from concourse import bass_utils

    nc = get_nc()
    in_maps = make_in_maps(inputs)
    res = bass_utils.run_bass_kernel_spmd(nc, in_maps, core_ids=list(range(8)))
    out = np.zeros((B, S, D), dtype=np.float32)
    for core in range(8):
        b = core // G
        out[b] += res.results[core]["y"]
    return out
